# revision 17
# baseline (speedup 1.0000x reference)
# kernel.py — Trainium2 Bass kernel for nn_ChannelAttentionBlock (v2.1)
#
# Computation (per reference):
#   h = relu(feature @ fc1_w.T + fc1_b)            [B,C,FF]
#   f = h @ fc2_w.T + fc2_b                        [B,C,HW]
#   T[b,n,m] = sum_c x[b,c,n] * f[b,c,m] * ls[m]   (ls = exp(min(logit_scale, log 100)))
#   P = softmax_n(T);  out[b,n,c] = sum_m P[n,m] x[b,c,m];  LayerNorm over c; -> [B,C,HW]
#
# Sharding (8 cores): MLP tensor-parallel on hidden; ReduceScatter over batch;
# attention data-parallel (core k = batch k).
#
# v2.1 structure (vs baseline):
#   - MLP weights: hi bf16 + lo fp8e4m3 (scaled 2^14) -> 25% less weight DMA;
#     lo matmuls accumulate in a second psum, combined+relu'd on DVE
#   - mm1 row-tiled 2x (K=24 strips at partitions 0/32)
#   - exp in [128,1024] calls, et kept fp32 in SBUF; Z via DVE reduce
#   - mm2 fp32r chains as baseline (fp32r cannot use PE sub-tiles)
#   - softmax shift via 3-tangent upper bound on ls*xmax*||f_m|| (no Sqrt =>
#     no ACT table switching); rstd = exp(-0.5*ln(var+eps))

import os
import numpy as np

B, C, HW, FF, P = 8, 6, 4096, 9216, 128
NCORES = 8
HS = FF // NCORES        # 1152
KT1 = FF // P            # 72 fc1 K tiles
KP1 = KT1 // 2           # 36 fc1 K-tile pairs
JT = HS // P             # 9  fc2 K tiles
NT = HW // 512           # 8  512-wide n/o chunks
MBS = HW // P            # 32 m blocks
NGRP = 4                 # reduce-scatter pipeline groups (2 o-chunks each)
EPS = 1e-5
HEADROOM = 35.0
U_CENTER = 0.578 ** 2    # E[||f_m||^2] from weight/feature statistics
U_TANGENTS = (U_CENTER * 0.16, U_CENTER, U_CENTER * 6.25)
NTAN = len(U_TANGENTS)
LO_SCALE = 2.0 ** 14     # fp8 lo-weight pre-scale

_cache = {}


def _build_program():
    import concourse.bacc as bacc
    import concourse.bass as bass
    import concourse.tile as tile
    import concourse.mybir as mybir

    dt = mybir.dt.float32
    dtr = mybir.dt.float32r
    dtb = mybir.dt.bfloat16
    dt8 = mybir.dt.float8e4
    AF = mybir.ActivationFunctionType
    ALU = mybir.AluOpType
    AX = mybir.AxisListType

    nc = bacc.Bacc(
        "TRN2",
        target_bir_lowering=False,
        debug=False,
        enable_asserts=False,
        num_devices=NCORES,
    )

    # ---- external I/O ----
    featT_d = nc.dram_tensor("featT", [P, KT1, 2, 48], dtb, kind="ExternalInput").ap()
    featT8_d = nc.dram_tensor("featT8", [P, KT1, 48], dt8, kind="ExternalInput").ap()
    w1h_d = nc.dram_tensor("w1h", [KT1 // 4, P, 4, HS], dtb, kind="ExternalInput").ap()
    w1l_d = nc.dram_tensor("w1l", [KT1 // 8, P, 8, HS], dt8, kind="ExternalInput").ap()
    b1_d = nc.dram_tensor("b1", [1, HS], dt, kind="ExternalInput").ap()
    w2h_d = nc.dram_tensor("w2h", [P, JT, HW], dtb, kind="ExternalInput").ap()
    w2l_d = nc.dram_tensor("w2l", [P, JT, HW], dt8, kind="ExternalInput").ap()
    b2_d = nc.dram_tensor("b2", [48, HW], dt, kind="ExternalInput").ap()  # fc2_b/8 bcast
    xaug_d = nc.dram_tensor("xaug", [24, HW], dtb, kind="ExternalInput").ap()
    xtb_d = nc.dram_tensor("xtb", [P, MBS, C], dt, kind="ExternalInput").ap()
    ls_d = nc.dram_tensor("lsb", [P, MBS], dt, kind="ExternalInput").ap()
    shA_d = nc.dram_tensor("shA", [P, NTAN, MBS], dt, kind="ExternalInput").ap()
    shB_d = nc.dram_tensor("shB", [P, NTAN, MBS], dt, kind="ExternalInput").ap()
    ones_d = nc.dram_tensor("ones1", [1, 48], dt, kind="ExternalInput").ap()
    ones6_d = nc.dram_tensor("ones6", [C, 1], dt, kind="ExternalInput").ap()
    id48_d = nc.dram_tensor("id48", [48, 48], dt, kind="ExternalInput").ap()
    blk_d = nc.dram_tensor("blk", [48, NT], dt, kind="ExternalInput").ap()
    blkT_d = nc.dram_tensor("blkT", [NT, 48], dt, kind="ExternalInput").ap()
    wb48_d = nc.dram_tensor("wb48", [48, 2], dt, kind="ExternalInput").ap()
    out_d = nc.dram_tensor("out", [C, HW], dt, kind="ExternalOutput").ap()

    with tile.TileContext(nc) as tc:
        # float32r APs carry full-fp32 bit patterns; the PE rounds at load.
        with nc.allow_low_precision(reason="fp32r/bf16/fp8 kernel dataflow"), \
             tc.tile_pool(name="const", bufs=1) as const, \
             tc.tile_pool(name="dram", bufs=1, space="DRAM") as dram:

            # ---- constants / small inputs ----
            xaug_sb = const.tile([24, HW], dtb, tag="xaug")
            nc.gpsimd.dma_start(out=xaug_sb[:], in_=xaug_d)
            xtb_sb = const.tile([P, MBS, C], dt, tag="xtb")
            nc.gpsimd.dma_start(out=xtb_sb[:], in_=xtb_d)
            ls_sb = const.tile([P, MBS], dt, tag="ls")
            nc.gpsimd.dma_start(out=ls_sb[:], in_=ls_d)
            shA_sb = const.tile([P, NTAN, MBS], dt, tag="shA")
            nc.gpsimd.dma_start(out=shA_sb[:], in_=shA_d)
            shB_sb = const.tile([P, NTAN, MBS], dt, tag="shB")
            nc.gpsimd.dma_start(out=shB_sb[:], in_=shB_d)
            ones_sb = const.tile([1, 48], dtr, tag="ones1")
            nc.gpsimd.dma_start(out=ones_sb[:], in_=ones_d.bitcast(dtr))
            ones6_sb = const.tile([C, 1], dt, tag="ones6")
            nc.gpsimd.dma_start(out=ones6_sb[:], in_=ones6_d)
            id48_sb = const.tile([48, 48], dt, tag="id48")
            nc.gpsimd.dma_start(out=id48_sb[:], in_=id48_d)
            blk_sb = const.tile([48, NT], dtr, tag="blk")
            nc.gpsimd.dma_start(out=blk_sb[:], in_=blk_d.bitcast(dtr))
            blkT_sb = const.tile([NT, 48], dtr, tag="blkT")
            nc.gpsimd.dma_start(out=blkT_sb[:], in_=blkT_d.bitcast(dtr))
            wb48_sb = const.tile([48, 2], dt, tag="wb48")
            nc.gpsimd.dma_start(out=wb48_sb[:], in_=wb48_d)

            b2s_sb = const.tile([48, HW], dt, tag="b2s")
            nc.gpsimd.dma_start(out=b2s_sb[:], in_=b2_d)
            hTh_sb = const.tile([P, JT, 48], dtb, tag="hTh")
            hTl_sb = const.tile([P, JT, 48], dtb, tag="hTl")
            hT8_sb = const.tile([P, JT, 48], dt8, tag="hT8")
            f_sb = const.tile([C, HW], dt, tag="f")
            faug_sb = const.tile([24, HW], dtb, tag="faug")
            shift_sb = const.tile([P, MBS], dt, tag="shift")
            O_nt = [const.tile([C, 512], dt, tag=f"O{nt}", name=f"O{nt}")
                    for nt in range(NT)]
            O48_sb = const.tile([48, 512], dt, tag="O48")

            rs_in = [dram.tile([48, 2, 512], dt, tag=f"rsin{g}", name=f"rsin{g}")
                     for g in range(NGRP)]
            rs_out = [dram.tile([C, 2, 512], dt, tag=f"rsout{g}", name=f"rsout{g}")
                      for g in range(NGRP)]

            # ================= MLP1: h = relu(feat @ w1 + b1) =================
            # hi: fh.wh + fl.wh (bf16); lo: fh8.(w1l*2^14) (fp8) in hp_lo
            with tc.tile_pool(name="w1p", bufs=3) as w1p, \
                 tc.tile_pool(name="w1lp", bufs=3) as w1lp, \
                 tc.tile_pool(name="m1c", bufs=1) as m1c, \
                 tc.tile_pool(name="ps1", bufs=1, space="PSUM") as ps1, \
                 tc.tile_pool(name="pst", bufs=2, space="PSUM") as pst:
                featT_sb = m1c.tile([P, KT1, 2, 48], dtb, tag="featT")
                nc.gpsimd.dma_start(out=featT_sb[:], in_=featT_d)
                featT8_sb = m1c.tile([P, KT1, 48], dt8, tag="featT8")
                nc.gpsimd.dma_start(out=featT8_sb[:], in_=featT8_d)
                b1_sb = m1c.tile([1, HS], dtr, tag="b1")
                nc.gpsimd.dma_start(out=b1_sb[:], in_=b1_d.bitcast(dtr))
                h_sb = m1c.tile([48, HS], dt, tag="h")
                tmp32 = m1c.tile([P, 48], dt, tag="tmp32")
                hp = ps1.tile([48, 3, 512], dt, tag="hp")
                hpl = ps1.tile([48, 3, 512], dt, tag="hpl")
                for kq in range(KT1 // 4):
                    w1t = w1p.tile([P, 4, HS], dtb, tag="w1t")
                    nc.sync.dma_start(out=w1t[:], in_=w1h_d[kq])
                    if kq % 2 == 0:
                        w1lt = w1lp.tile([P, 8, HS], dt8, tag="w1lt")
                        nc.sync.dma_start(out=w1lt[:], in_=w1l_d[kq // 2])
                    for s in range(4):
                        kk = 4 * kq + s
                        fh = featT_sb[:, kk, 0, :]
                        fl = featT_sb[:, kk, 1, :]
                        f8 = featT8_sb[:, kk, :]
                        for j in range(3):
                            jsl = slice(j * 384, (j + 1) * 384)
                            nc.tensor.matmul(
                                hp[:, j, 0:384], lhsT=fh, rhs=w1t[:, s, jsl],
                                start=(kk == 0), stop=False)
                            nc.tensor.matmul(
                                hp[:, j, 0:384], lhsT=fl, rhs=w1t[:, s, jsl],
                                start=False, stop=False)
                            nc.tensor.matmul(
                                hpl[:, j, 0:384], lhsT=f8,
                                rhs=w1lt[:, (kq % 2) * 4 + s, jsl],
                                start=(kk == 0),
                                stop=(kk == KT1 - 1))
                for j in range(3):  # bias via K=1 ones row
                    nc.tensor.matmul(
                        hp[:, j, 0:384],
                        lhsT=ones_sb[:],
                        rhs=b1_sb[:, j * 384:(j + 1) * 384],
                        start=False,
                        stop=True,
                    )
                for j in range(3):  # combine lo (descale) + relu on DVE
                    jsl = slice(j * 384, (j + 1) * 384)
                    hl_sb = m1c.tile([48, 384], dt, tag=f"hl{j}")
                    nc.vector.tensor_scalar_mul(
                        hl_sb[:], hpl[:, j, 0:384], 1.0 / LO_SCALE)
                    nc.vector.tensor_add(h_sb[:, jsl], hl_sb[:], hp[:, j, 0:384])
                    nc.vector.tensor_relu(h_sb[:, jsl], h_sb[:, jsl])
                # transpose h -> hT (9 PE transposes of [48,128]), split hi/lo/fp8
                for t in range(JT):
                    tp = pst.tile([P, 48], dt, tag="tp")
                    nc.tensor.transpose(
                        tp[:], h_sb[:, t * P:(t + 1) * P], id48_sb[:]
                    )
                    nc.vector.tensor_copy(hTh_sb[:, t, :], tp[:])
                    nc.vector.tensor_sub(tmp32[:], tp[:], hTh_sb[:, t, :])
                    nc.vector.tensor_copy(hTl_sb[:, t, :], tmp32[:])
                    nc.vector.tensor_copy(hT8_sb[:, t, :], hTh_sb[:, t, :])

            # ============ MLP2 + ReduceScatter + attention (pipelined) ============
            with tc.tile_pool(name="w2p", bufs=2) as w2p, \
                 tc.tile_pool(name="w2lp", bufs=2) as w2lp, \
                 tc.tile_pool(name="fpe", bufs=2) as fpep, \
                 tc.tile_pool(name="fsp", bufs=2) as fsp, \
                 tc.tile_pool(name="mps", bufs=2, space="PSUM") as mps, \
                 tc.tile_pool(name="tpp", bufs=2, space="PSUM") as tpp, \
                 tc.tile_pool(name="cps", bufs=2, space="PSUM") as cpsp, \
                 tc.tile_pool(name="etp", bufs=3) as etp, \
                 tc.tile_pool(name="xpp", bufs=4) as xpp:

                first_flush = [True] * NT

                def emit_mlp2_group(g):
                    for occ in range(2):
                        oc = 2 * g + occ
                        osl = slice(oc * 512, (oc + 1) * 512)
                        w2t = w2p.tile([P, JT, 512], dtb, tag="w2t")
                        nc.sync.dma_start(out=w2t[:], in_=w2h_d[:, :, osl])
                        w2lt = w2lp.tile([P, JT, 512], dt8, tag="w2lt")
                        nc.sync.dma_start(out=w2lt[:], in_=w2l_d[:, :, osl])
                        fp = mps.tile([48, 512], dt, tag="fp")
                        fpl = mps.tile([48, 512], dt, tag="fp")
                        for jj in range(JT):
                            nc.tensor.matmul(
                                fp[:], lhsT=hTh_sb[:, jj, :], rhs=w2t[:, jj, :],
                                start=(jj == 0), stop=False)
                            nc.tensor.matmul(
                                fp[:], lhsT=hTl_sb[:, jj, :], rhs=w2t[:, jj, :],
                                start=False, stop=(jj == JT - 1))
                            nc.tensor.matmul(
                                fpl[:], lhsT=hT8_sb[:, jj, :], rhs=w2lt[:, jj, :],
                                start=(jj == 0), stop=(jj == JT - 1))
                        fle = fpep.tile([48, 512], dt, tag="fle")
                        nc.vector.scalar_tensor_tensor(
                            out=fle[:], in0=fpl[:], scalar=1.0 / LO_SCALE,
                            in1=b2s_sb[:, osl], op0=ALU.mult, op1=ALU.add)
                        fpe = fpep.tile([48, 512], dt, tag="fpe")
                        nc.vector.tensor_add(fpe[:], fle[:], fp[:])
                        nc.sync.dma_start(out=rs_in[g][:, occ, :], in_=fpe[:])
                    nc.gpsimd.collective_compute(
                        "ReduceScatter",
                        ALU.add,
                        replica_groups=[list(range(NCORES))],
                        ins=[rs_in[g].opt()],
                        outs=[rs_out[g].opt()],
                    )
                    gsl = slice(g * 1024, (g + 1) * 1024)
                    nc.sync.dma_start(out=f_sb[:, gsl], in_=rs_out[g][:])
                    # bf16 hi/lo split of f for mm1, rows [fh,fh,fl,fl] x2 strips
                    fh = fsp.tile([C, 1024], dtb, tag="fh")
                    nc.vector.tensor_copy(fh[:], f_sb[:, gsl])
                    fl32 = fsp.tile([C, 1024], dt, tag="fl32")
                    nc.vector.tensor_sub(fl32[:], f_sb[:, gsl], fh[:])
                    fl = fsp.tile([C, 1024], dtb, tag="fl")
                    nc.vector.tensor_copy(fl[:], fl32[:])
                    nc.sync.dma_start(out=faug_sb[0:C, gsl], in_=fh[:])
                    nc.sync.dma_start(out=faug_sb[C:2 * C, gsl], in_=fh[:])
                    nc.sync.dma_start(out=faug_sb[2 * C:3 * C, gsl], in_=fl[:])
                    nc.sync.dma_start(out=faug_sb[3 * C:24, gsl], in_=fl[:])
                    # shift[m]: 3-tangent upper bound from u = ||f_m||^2
                    f2 = fsp.tile([C, 1024], dt, tag="f2")
                    nc.vector.tensor_mul(f2[:], f_sb[:, gsl], f_sb[:, gsl])
                    up = mps.tile([P, 8], dt, tag="fp")
                    for j in range(8):
                        nc.tensor.matmul(
                            up[:, j:j + 1],
                            lhsT=f2[:, j * P:(j + 1) * P],
                            rhs=ones6_sb[:],
                            start=True,
                            stop=True,
                        )
                    g8 = slice(g * 8, (g + 1) * 8)
                    t1 = fsp.tile([P, 8], dt, tag="t1")
                    t2 = fsp.tile([P, 8], dt, tag="t2")
                    nc.vector.tensor_mul(t1[:], up[:], shA_sb[:, 0, g8])
                    nc.vector.tensor_add(t1[:], t1[:], shB_sb[:, 0, g8])
                    for i in range(1, NTAN):
                        nc.vector.tensor_mul(t2[:], up[:], shA_sb[:, i, g8])
                        nc.vector.tensor_add(t2[:], t2[:], shB_sb[:, i, g8])
                        nc.vector.tensor_max(t1[:], t1[:], t2[:])
                    nc.vector.tensor_copy(shift_sb[:, g8], t1[:])

                pending = []  # software-pipelined mm2 (pairs of m-blocks)

                def emit_mm2_pair(pair):
                    for nt in range(NT):
                        cp = cpsp.tile([C, 512], dt, tag="cp")
                        for i, (mb, et, xp) in enumerate(pair):
                            nc.tensor.matmul(
                                cp[:],
                                lhsT=xp[:].bitcast(dtr),
                                rhs=et[:, nt // 2, nt % 2, :].bitcast(dtr),
                                start=(i == 0),
                                stop=(i == len(pair) - 1),
                            )
                        if first_flush[nt]:
                            nc.vector.tensor_copy(O_nt[nt][:], cp[:])
                            first_flush[nt] = False
                        else:
                            nc.vector.tensor_add(O_nt[nt][:], O_nt[nt][:], cp[:])

                def emit_att_mblock(mb):
                    et = etp.tile([P, 4, 2, 512], dt, tag="et")
                    lhs0 = faug_sb[:, mb * P:(mb + 1) * P]
                    for r in range(4):  # 2 n-chunks per psum tile
                        tps = tpp.tile([P, 2, 512], dt, tag="tps")
                        for i in range(2):
                            nt = 2 * r + i
                            nc.tensor.matmul(
                                tps[:, i, :],
                                lhsT=lhs0,
                                rhs=xaug_sb[:, nt * 512:(nt + 1) * 512],
                                start=True,
                                stop=True,
                            )
                        nc.scalar.activation(
                            et[:, r, :, :].bitcast(dtr),
                            tps[:],
                            AF.Exp,
                            scale=ls_sb[:, mb:mb + 1],
                            bias=shift_sb[:, mb:mb + 1],
                        )
                    # Z and xp on DVE
                    zz = xpp.tile([P, 1], dt, tag="zz")
                    nc.vector.tensor_reduce(zz[:], et[:], AX.XYZ, ALU.add)
                    rc = xpp.tile([P, 1], dt, tag="rc")
                    nc.vector.reciprocal(rc[:], zz[:])
                    xp = xpp.tile([P, C], dt, tag="xp")
                    nc.vector.tensor_scalar_mul(
                        xp[:].bitcast(dtr), xtb_sb[:, mb, :], rc[:])
                    pending.append((mb, et, xp))
                    if len(pending) == 3:
                        emit_mm2_pair(pending[:2])
                        del pending[:2]

                for g in range(NGRP):
                    emit_mlp2_group(g)
                    if g > 0:
                        for mb in range((g - 1) * NT, g * NT):
                            emit_att_mblock(mb)
                for mb in range((NGRP - 1) * NT, NGRP * NT):
                    emit_att_mblock(mb)
                while pending:
                    emit_mm2_pair(pending[:2])
                    del pending[:2]

                # stack the 8 [6,512] chunks into [48,512]
                for nt in range(NT):
                    nc.sync.dma_start(
                        out=O48_sb[C * nt:C * nt + C, :].bitcast(dtr),
                        in_=O_nt[nt][:].bitcast(dtr))

            # ===================== LayerNorm over c + output =====================
            with tc.tile_pool(name="lnps", bufs=2, space="PSUM") as lnps, \
                 tc.tile_pool(name="lnrp", bufs=2, space="PSUM") as lnrp, \
                 tc.tile_pool(name="lnsb", bufs=1) as lnsb:
                eps_sb = lnsb.tile([NT, 1], dt, tag="eps")
                nc.vector.memset(eps_sb[:], EPS)
                O2_sb = lnsb.tile([48, 512], dt, tag="O2")
                nc.vector.tensor_mul(O2_sb[:].bitcast(dtr), O48_sb[:], O48_sb[:])
                s_ps = lnps.tile([NT, 512], dt, tag="sps")
                nc.tensor.matmul(
                    s_ps[:], lhsT=blk_sb[:], rhs=O48_sb[:].bitcast(dtr),
                    start=True, stop=True,
                )
                s2_ps = lnps.tile([NT, 512], dt, tag="sps")
                nc.tensor.matmul(
                    s2_ps[:], lhsT=blk_sb[:], rhs=O2_sb[:].bitcast(dtr),
                    start=True, stop=True,
                )
                mean_sb = lnsb.tile([NT, 512], dt, tag="mean")
                nc.vector.tensor_scalar_mul(
                    mean_sb[:].bitcast(dtr), s_ps[:], 1.0 / C)
                ms_sb = lnsb.tile([NT, 512], dt, tag="ms")
                nc.vector.tensor_mul(ms_sb[:], mean_sb[:], mean_sb[:])
                var_sb = lnsb.tile([NT, 512], dt, tag="var")
                nc.vector.tensor_scalar_mul(var_sb[:], s2_ps[:], 1.0 / C)
                nc.vector.tensor_sub(var_sb[:], var_sb[:], ms_sb[:])
                # rstd = exp(-0.5*ln(var+eps)); Ln+Exp share an ACT table set
                lv_sb = lnsb.tile([NT, 512], dt, tag="lv")
                nc.scalar.activation(lv_sb[:], var_sb[:], AF.Ln, bias=eps_sb[:])
                rstd_sb = lnsb.tile([NT, 512], dt, tag="rstd")
                nc.scalar.activation(rstd_sb[:].bitcast(dtr), lv_sb[:],
                                     AF.Exp, scale=-0.5)
                mrep = lnrp.tile([48, 512], dt, tag="mrep")
                nc.tensor.matmul(
                    mrep[:], lhsT=blkT_sb[:], rhs=mean_sb[:].bitcast(dtr),
                    start=True, stop=True,
                )
                rrep = lnrp.tile([48, 512], dt, tag="mrep")
                nc.tensor.matmul(
                    rrep[:], lhsT=blkT_sb[:], rhs=rstd_sb[:].bitcast(dtr),
                    start=True, stop=True,
                )
                on_sb = lnsb.tile([48, 512], dt, tag="on")
                nc.vector.tensor_sub(on_sb[:], O48_sb[:], mrep[:])
                nc.vector.tensor_mul(on_sb[:], on_sb[:], rrep[:])
                nc.vector.tensor_scalar(
                    out=on_sb[:], in0=on_sb[:],
                    scalar1=wb48_sb[:, 0:1], scalar2=wb48_sb[:, 1:2],
                    op0=ALU.mult, op1=ALU.add,
                )
                for nt in range(NT):
                    nc.sync.dma_start(
                        out=out_d[:, nt * 512:(nt + 1) * 512],
                        in_=on_sb[C * nt:C * nt + C, :],
                    )

    nc.compile()
    return nc


def _host_prep(inputs):
    import ml_dtypes
    bf16 = ml_dtypes.bfloat16
    f8 = ml_dtypes.float8_e4m3

    x = np.asarray(inputs["x"], np.float32)
    feature = np.asarray(inputs["feature"], np.float32)
    fc1_w = np.asarray(inputs["fc1_w"], np.float32)
    fc1_b = np.asarray(inputs["fc1_b"], np.float32)
    fc2_w = np.asarray(inputs["fc2_w"], np.float32)
    fc2_b = np.asarray(inputs["fc2_b"], np.float32)
    logit_scale = np.asarray(inputs["logit_scale"], np.float32)
    norm_w = np.asarray(inputs["norm_w"], np.float32)
    norm_b = np.asarray(inputs["norm_b"], np.float32)

    def split_hl(a):
        hi = a.astype(bf16)
        lo = (a - hi.astype(np.float32)).astype(bf16)
        return hi, lo

    def split_hl8(a):
        hi = a.astype(bf16)
        lo = ((a - hi.astype(np.float32)) * LO_SCALE).astype(f8)
        return hi, lo

    w1T = np.ascontiguousarray(fc1_w.T)                      # [f, h]
    w2T = np.ascontiguousarray(fc2_w.T)                      # [h, o]
    featT = np.ascontiguousarray(feature.reshape(B * C, FF).T)   # [f, bc]
    fth, ftl = split_hl(featT)
    featT_b = np.ascontiguousarray(
        np.stack([fth, ftl], axis=1).reshape(KT1, P, 2, B * C)
        .transpose(1, 0, 2, 3))
    featT_8 = np.ascontiguousarray(
        fth.astype(f8).reshape(KT1, P, B * C).transpose(1, 0, 2))
    ls = np.exp(np.minimum(logit_scale.reshape(HW), np.log(np.float32(100.0))))
    ls_b = np.ascontiguousarray(ls.reshape(MBS, P).T).astype(np.float32)
    ones1 = np.ones((1, B * C), np.float32)
    ones6 = np.ones((C, 1), np.float32)
    id48 = np.eye(48, dtype=np.float32)
    blk = np.zeros((48, NT), np.float32)
    blk[np.arange(48), np.arange(48) // C] = 1.0
    blkT = np.ascontiguousarray(blk.T)
    wb48 = np.ascontiguousarray(
        np.stack([np.tile(norm_w, NT), np.tile(norm_b, NT)], axis=1))
    b2 = np.tile((fc2_b / NCORES).reshape(1, HW), (48, 1)).astype(np.float32)

    in_maps = []
    for k in range(NCORES):
        w1k = np.ascontiguousarray(w1T[:, k * HS:(k + 1) * HS])   # [9216, 1152]
        w1h, w1l = split_hl8(w1k)
        w1hs = np.ascontiguousarray(
            w1h.reshape(KT1 // 4, 4, P, HS).transpose(0, 2, 1, 3))
        w1ls = np.ascontiguousarray(
            w1l.reshape(KT1 // 8, 8, P, HS).transpose(0, 2, 1, 3))
        b1k = np.ascontiguousarray(fc1_b[k * HS:(k + 1) * HS]).reshape(1, HS)
        w2k = np.ascontiguousarray(w2T[k * HS:(k + 1) * HS, :])   # [1152, 4096]
        w2h, w2l = split_hl8(w2k)
        w2hs = np.ascontiguousarray(w2h.reshape(JT, P, HW).transpose(1, 0, 2))
        w2ls = np.ascontiguousarray(w2l.reshape(JT, P, HW).transpose(1, 0, 2))
        xh, xl = split_hl(x[k])                                   # [6, 4096]
        xaug = np.concatenate([xh, xl, xh, xl], axis=0)           # [24, 4096]

        xtbk = np.ascontiguousarray(x[k].T.reshape(MBS, P, C).transpose(1, 0, 2))
        xmax_k = float(np.linalg.norm(x[k], axis=0).max())
        # tangent-line bounds on ls*xmax*sqrt(u) at u = U_TANGENTS[i]:
        #   bound_i(u) = ls*xmax*(u/sqrt(u_i) + sqrt(u_i))/2
        # shift = max_i(-bound_i(u)) + HEADROOM = max_i(A_i*u + B_i)
        shA = np.zeros((P, NTAN, MBS), np.float32)
        shB = np.zeros((P, NTAN, MBS), np.float32)
        for i, u0 in enumerate(U_TANGENTS):
            r = np.sqrt(np.float32(u0))
            shA[:, i, :] = -ls_b * xmax_k / (2 * r)
            shB[:, i, :] = -ls_b * xmax_k * r / 2 + HEADROOM
        in_maps.append({
            "featT": featT_b, "featT8": featT_8,
            "w1h": w1hs, "w1l": w1ls, "b1": b1k,
            "w2h": w2hs, "w2l": w2ls, "b2": b2,
            "xaug": np.ascontiguousarray(xaug), "xtb": xtbk, "lsb": ls_b, "shA": shA, "shB": shB,
            "ones1": ones1, "ones6": ones6, "id48": id48,
            "blk": blk, "blkT": blkT, "wb48": wb48,
        })
    return in_maps


def _install_ntff_shim():
    # The agent image's `antenv` lacks `axon_hooks`, which bass_utils needs
    # for trace=True under axon. Fabricate the registry module and install
    # the ctypes-based NTFF hook against libaxon_pjrt.so.
    import sys
    import types
    import ctypes
    import contextlib

    try:
        import antenv.axon_hooks  # noqa: F401
        return
    except ImportError:
        pass
    if "antenv.axon_hooks" in sys.modules:
        return
    mod = types.ModuleType("antenv.axon_hooks")
    _h = [None]
    mod.set_axon_ntff_profile_hook = lambda h: _h.__setitem__(0, h)
    mod.get_axon_ntff_profile_hook = lambda: _h[0]
    sys.modules["antenv.axon_hooks"] = mod

    so_path = "/opt/axon/libaxon_pjrt.so"
    if not os.path.exists(so_path):
        return
    lib = ctypes.CDLL(so_path)
    if not hasattr(lib, "axon_start_nrt_profile"):
        return
    lib.axon_start_nrt_profile.argtypes = [
        ctypes.POINTER(ctypes.c_int64), ctypes.c_size_t]
    lib.axon_start_nrt_profile.restype = ctypes.c_int64
    lib.axon_stop_nrt_profile.argtypes = [ctypes.c_char_p]
    lib.axon_stop_nrt_profile.restype = ctypes.c_int64

    @contextlib.contextmanager
    def _hook(output_dir, device_ids):
        import jax
        jax.devices()
        if device_ids:
            ids = (ctypes.c_int64 * len(device_ids))(*device_ids)
            rc = lib.axon_start_nrt_profile(ids, len(device_ids))
        else:
            rc = lib.axon_start_nrt_profile(None, 0)
        if rc != 0:
            raise RuntimeError(f"axon_start_nrt_profile rc={rc}")
        try:
            yield
        finally:
            n = lib.axon_stop_nrt_profile(str(output_dir).encode())
            print(f"ntff profile: {n} file(s) written to {output_dir}")

    mod.set_axon_ntff_profile_hook(_hook)


def kernel(**inputs):
    from concourse.bass_utils import run_bass_kernel_spmd

    if bool(int(os.environ.get("BASS_KT_TRACE", "0"))):
        _install_ntff_shim()

    if "nc" not in _cache:
        _cache["nc"] = _build_program()
    nc = _cache["nc"]

    in_maps = _host_prep(inputs)
    trace = bool(int(os.environ.get("BASS_KT_TRACE", "0")))
    res = run_bass_kernel_spmd(nc, in_maps, list(range(NCORES)), trace=trace)
    kernel.last_results = res
    out = np.stack([np.asarray(res.results[k]["out"]) for k in range(NCORES)])
    return out.astype(np.float32)


# revision 19
# speedup vs baseline: 1.2031x; 1.2031x over previous
# kernel.py — Trainium2 Bass kernel for nn_ChannelAttentionBlock (v2.1)
#
# Computation (per reference):
#   h = relu(feature @ fc1_w.T + fc1_b)            [B,C,FF]
#   f = h @ fc2_w.T + fc2_b                        [B,C,HW]
#   T[b,n,m] = sum_c x[b,c,n] * f[b,c,m] * ls[m]   (ls = exp(min(logit_scale, log 100)))
#   P = softmax_n(T);  out[b,n,c] = sum_m P[n,m] x[b,c,m];  LayerNorm over c; -> [B,C,HW]
#
# Sharding (8 cores): MLP tensor-parallel on hidden; ReduceScatter over batch;
# attention data-parallel (core k = batch k).
#
# v2.1 structure (vs baseline):
#   - MLP weights: hi bf16 + lo fp8e4m3 (scaled 2^14) -> 25% less weight DMA;
#     lo matmuls accumulate in a second psum, combined+relu'd on DVE
#   - mm1 row-tiled 2x (K=24 strips at partitions 0/32)
#   - exp in [128,1024] calls, et kept fp32 in SBUF; Z via DVE reduce
#   - mm2 fp32r chains as baseline (fp32r cannot use PE sub-tiles)
#   - softmax shift via 3-tangent upper bound on ls*xmax*||f_m|| (no Sqrt =>
#     no ACT table switching); rstd = exp(-0.5*ln(var+eps))

import os
import numpy as np

B, C, HW, FF, P = 8, 6, 4096, 9216, 128
NCORES = 8
HS = FF // NCORES        # 1152
KT1 = FF // P            # 72 fc1 K tiles
KP1 = KT1 // 2           # 36 fc1 K-tile pairs
JT = HS // P             # 9  fc2 K tiles
NT = HW // 512           # 8  512-wide n/o chunks
MBS = HW // P            # 32 m blocks
NGRP = 8                 # reduce-scatter pipeline groups (1 o-chunk each)
EPS = 1e-5
HEADROOM = 35.0
U_CENTER = 0.578 ** 2    # E[||f_m||^2] from weight/feature statistics
U_TANGENTS = (U_CENTER * 0.16, U_CENTER, U_CENTER * 6.25)
NTAN = len(U_TANGENTS)
LO_SCALE = 2.0 ** 14     # fp8 lo-weight pre-scale

_cache = {}


def _build_program():
    import concourse.bacc as bacc
    import concourse.bass as bass
    import concourse.tile as tile
    import concourse.mybir as mybir

    dt = mybir.dt.float32
    dtr = mybir.dt.float32r
    dtb = mybir.dt.bfloat16
    dt8 = mybir.dt.float8e4
    AF = mybir.ActivationFunctionType
    ALU = mybir.AluOpType
    AX = mybir.AxisListType

    nc = bacc.Bacc(
        "TRN2",
        target_bir_lowering=False,
        debug=False,
        enable_asserts=False,
        num_devices=NCORES,
    )

    # ---- external I/O ----
    featT_d = nc.dram_tensor("featT", [P, KT1, 2, 48], dtb, kind="ExternalInput").ap()
    featT8_d = nc.dram_tensor("featT8", [P, KT1, 48], dt8, kind="ExternalInput").ap()
    w1h_d = nc.dram_tensor("w1h", [KT1 // 4, P, 4, HS], dtb, kind="ExternalInput").ap()
    w1l_d = nc.dram_tensor("w1l", [KT1 // 8, P, 8, HS], dt8, kind="ExternalInput").ap()
    b1_d = nc.dram_tensor("b1", [1, HS], dt, kind="ExternalInput").ap()
    w2h_d = nc.dram_tensor("w2h", [P, JT, HW], dtb, kind="ExternalInput").ap()
    w2l_d = nc.dram_tensor("w2l", [P, JT, HW], dt8, kind="ExternalInput").ap()
    b2_d = nc.dram_tensor("b2", [48, HW], dt, kind="ExternalInput").ap()  # fc2_b/8 bcast
    xaug_d = nc.dram_tensor("xaug", [24, HW], dtb, kind="ExternalInput").ap()
    xtb_d = nc.dram_tensor("xtb", [P, MBS, C], dt, kind="ExternalInput").ap()
    ls_d = nc.dram_tensor("lsb", [P, MBS], dt, kind="ExternalInput").ap()
    shA_d = nc.dram_tensor("shA", [P, NTAN, MBS], dt, kind="ExternalInput").ap()
    shB_d = nc.dram_tensor("shB", [P, NTAN, MBS], dt, kind="ExternalInput").ap()
    ones_d = nc.dram_tensor("ones1", [1, 48], dt, kind="ExternalInput").ap()
    ones6_d = nc.dram_tensor("ones6", [C, 1], dtb, kind="ExternalInput").ap()
    id48_d = nc.dram_tensor("id48", [48, 48], dt, kind="ExternalInput").ap()
    blk_d = nc.dram_tensor("blk", [48, NT], dt, kind="ExternalInput").ap()
    blkT_d = nc.dram_tensor("blkT", [NT, 48], dt, kind="ExternalInput").ap()
    wb48_d = nc.dram_tensor("wb48", [48, 2], dt, kind="ExternalInput").ap()
    out_d = nc.dram_tensor("out", [C, HW], dt, kind="ExternalOutput").ap()

    with tile.TileContext(nc) as tc:
        # float32r APs carry full-fp32 bit patterns; the PE rounds at load.
        with nc.allow_low_precision(reason="fp32r/bf16/fp8 kernel dataflow"), \
             tc.tile_pool(name="const", bufs=1) as const, \
             tc.tile_pool(name="dram", bufs=1, space="DRAM") as dram:

            # ---- constants / small inputs ----
            xaug_sb = const.tile([24, HW], dtb, tag="xaug")
            nc.gpsimd.dma_start(out=xaug_sb[:], in_=xaug_d)
            xtb_sb = const.tile([P, MBS, C], dt, tag="xtb")
            nc.gpsimd.dma_start(out=xtb_sb[:], in_=xtb_d)
            ls_sb = const.tile([P, MBS], dt, tag="ls")
            nc.gpsimd.dma_start(out=ls_sb[:], in_=ls_d)
            shA_sb = const.tile([P, NTAN, MBS], dt, tag="shA")
            nc.gpsimd.dma_start(out=shA_sb[:], in_=shA_d)
            shB_sb = const.tile([P, NTAN, MBS], dt, tag="shB")
            nc.gpsimd.dma_start(out=shB_sb[:], in_=shB_d)
            ones_sb = const.tile([1, 48], dtr, tag="ones1")
            nc.gpsimd.dma_start(out=ones_sb[:], in_=ones_d.bitcast(dtr))
            ones6_sb = const.tile([C, 1], dtb, tag="ones6")
            nc.gpsimd.dma_start(out=ones6_sb[:], in_=ones6_d)
            id48_sb = const.tile([48, 48], dt, tag="id48")
            nc.gpsimd.dma_start(out=id48_sb[:], in_=id48_d)
            blk_sb = const.tile([48, NT], dtr, tag="blk")
            nc.gpsimd.dma_start(out=blk_sb[:], in_=blk_d.bitcast(dtr))
            blkT_sb = const.tile([NT, 48], dtr, tag="blkT")
            nc.gpsimd.dma_start(out=blkT_sb[:], in_=blkT_d.bitcast(dtr))
            wb48_sb = const.tile([48, 2], dt, tag="wb48")
            nc.gpsimd.dma_start(out=wb48_sb[:], in_=wb48_d)

            b2s_sb = const.tile([48, HW], dt, tag="b2s")
            nc.gpsimd.dma_start(out=b2s_sb[:], in_=b2_d)
            hTh_sb = const.tile([P, JT, 48], dtb, tag="hTh")
            hTl_sb = const.tile([P, JT, 48], dtb, tag="hTl")
            hT8_sb = const.tile([P, JT, 48], dt8, tag="hT8")
            f_sb = const.tile([C, HW], dt, tag="f")
            faug_sb = const.tile([24, HW], dtb, tag="faug")
            shift_sb = const.tile([P, MBS], dt, tag="shift")
            O_nt = [const.tile([C, 512], dt, tag=f"O{nt}", name=f"O{nt}")
                    for nt in range(NT)]
            O48_sb = const.tile([48, 512], dt, tag="O48")

            rs_in = [dram.tile([48, 512], dt, tag=f"rsin{g}", name=f"rsin{g}")
                     for g in range(NGRP)]
            rs_out = [dram.tile([C, 512], dt, tag=f"rsout{g}", name=f"rsout{g}")
                      for g in range(NGRP)]
            warm_in = dram.tile([8, 8], dt, tag="warmin", name="warmin")
            warm_out = dram.tile([1, 8], dt, tag="warmout", name="warmout")
            warm_sb = const.tile([8, 8], dt, tag="warm")
            nc.vector.memset(warm_sb[:], 0.0)
            nc.sync.dma_start(out=warm_in[:], in_=warm_sb[:])
            nc.gpsimd.collective_compute(
                "ReduceScatter", ALU.add,
                replica_groups=[list(range(NCORES))],
                ins=[warm_in.opt()], outs=[warm_out.opt()])

            # ================= MLP1: h = relu(feat @ w1 + b1) =================
            # hi: fh.wh + fl.wh (bf16); lo: fh8.(w1l*2^14) (fp8) in hp_lo
            with tc.tile_pool(name="w1p", bufs=3) as w1p, \
                 tc.tile_pool(name="w1lp", bufs=3) as w1lp, \
                 tc.tile_pool(name="m1c", bufs=1) as m1c, \
                 tc.tile_pool(name="ps1", bufs=1, space="PSUM") as ps1, \
                 tc.tile_pool(name="pst", bufs=2, space="PSUM") as pst:
                featT_sb = m1c.tile([P, KT1, 2, 48], dtb, tag="featT")
                nc.sync.dma_start(out=featT_sb[:], in_=featT_d)
                featT8_sb = m1c.tile([P, KT1, 48], dt8, tag="featT8")
                nc.sync.dma_start(out=featT8_sb[:], in_=featT8_d)
                b1_sb = m1c.tile([1, HS], dtr, tag="b1")
                nc.gpsimd.dma_start(out=b1_sb[:], in_=b1_d.bitcast(dtr))
                h_sb = m1c.tile([48, HS], dt, tag="h")
                tmp32 = m1c.tile([P, 48], dt, tag="tmp32")
                hp = ps1.tile([48, 3, 512], dt, tag="hp")
                hpl = ps1.tile([48, 3, 512], dt, tag="hpl")
                for kq in range(KT1 // 4):
                    w1t = w1p.tile([P, 4, HS], dtb, tag="w1t")
                    nc.sync.dma_start(out=w1t[:], in_=w1h_d[kq])
                    if kq % 2 == 0:
                        w1lt = w1lp.tile([P, 8, HS], dt8, tag="w1lt")
                        nc.sync.dma_start(out=w1lt[:], in_=w1l_d[kq // 2])
                    for s in range(4):
                        kk = 4 * kq + s
                        fh = featT_sb[:, kk, 0, :]
                        fl = featT_sb[:, kk, 1, :]
                        f8 = featT8_sb[:, kk, :]
                        for j in range(3):
                            jsl = slice(j * 384, (j + 1) * 384)
                            nc.tensor.matmul(
                                hp[:, j, 0:384], lhsT=fh, rhs=w1t[:, s, jsl],
                                start=(kk == 0), stop=False)
                            nc.tensor.matmul(
                                hp[:, j, 0:384], lhsT=fl, rhs=w1t[:, s, jsl],
                                start=False, stop=False)
                            nc.tensor.matmul(
                                hpl[:, j, 0:384], lhsT=f8,
                                rhs=w1lt[:, (kq % 2) * 4 + s, jsl],
                                start=(kk == 0),
                                stop=(kk == KT1 - 1))
                for j in range(3):  # bias via K=1 ones row
                    nc.tensor.matmul(
                        hp[:, j, 0:384],
                        lhsT=ones_sb[:],
                        rhs=b1_sb[:, j * 384:(j + 1) * 384],
                        start=False,
                        stop=True,
                    )
                for j in range(3):  # combine lo (descale) + relu on DVE
                    jsl = slice(j * 384, (j + 1) * 384)
                    hl_sb = m1c.tile([48, 384], dt, tag=f"hl{j}")
                    nc.vector.tensor_scalar_mul(
                        hl_sb[:], hpl[:, j, 0:384], 1.0 / LO_SCALE)
                    nc.vector.tensor_add(h_sb[:, jsl], hl_sb[:], hp[:, j, 0:384])
                    nc.vector.tensor_relu(h_sb[:, jsl], h_sb[:, jsl])
                # transpose h -> hT (9 PE transposes of [48,128]), split hi/lo/fp8
                for t in range(JT):
                    tp = pst.tile([P, 48], dt, tag="tp")
                    nc.tensor.transpose(
                        tp[:], h_sb[:, t * P:(t + 1) * P], id48_sb[:]
                    )
                    nc.vector.tensor_copy(hTh_sb[:, t, :], tp[:])
                    nc.vector.tensor_sub(tmp32[:], tp[:], hTh_sb[:, t, :])
                    nc.vector.tensor_copy(hTl_sb[:, t, :], tmp32[:])
                    nc.vector.tensor_copy(hT8_sb[:, t, :], hTh_sb[:, t, :])

            # ============ MLP2 + ReduceScatter + attention (pipelined) ============
            with tc.tile_pool(name="w2p", bufs=2) as w2p, \
                 tc.tile_pool(name="w2lp", bufs=2) as w2lp, \
                 tc.tile_pool(name="fpe", bufs=2) as fpep, \
                 tc.tile_pool(name="fsp", bufs=2) as fsp, \
                 tc.tile_pool(name="mps", bufs=2, space="PSUM") as mps, \
                 tc.tile_pool(name="tpp", bufs=2, space="PSUM") as tpp, \
                 tc.tile_pool(name="cps", bufs=2, space="PSUM") as cpsp, \
                 tc.tile_pool(name="etp", bufs=5) as etp, \
                 tc.tile_pool(name="xpp", bufs=6) as xpp:

                first_flush = [True] * NT

                def emit_mlp2_group(g):
                    oc = g
                    osl = slice(oc * 512, (oc + 1) * 512)
                    w2t = w2p.tile([P, JT, 512], dtb, tag="w2t")
                    nc.sync.dma_start(out=w2t[:], in_=w2h_d[:, :, osl])
                    w2lt = w2lp.tile([P, JT, 512], dt8, tag="w2lt")
                    nc.sync.dma_start(out=w2lt[:], in_=w2l_d[:, :, osl])
                    fp = mps.tile([48, 512], dt, tag="fp")
                    fpl = mps.tile([48, 512], dt, tag="fp")
                    for jj in range(JT):
                        nc.tensor.matmul(
                            fp[:], lhsT=hTh_sb[:, jj, :], rhs=w2t[:, jj, :],
                            start=(jj == 0), stop=False)
                        nc.tensor.matmul(
                            fp[:], lhsT=hTl_sb[:, jj, :], rhs=w2t[:, jj, :],
                            start=False, stop=(jj == JT - 1))
                        nc.tensor.matmul(
                            fpl[:], lhsT=hT8_sb[:, jj, :], rhs=w2lt[:, jj, :],
                            start=(jj == 0), stop=(jj == JT - 1))
                    fle = fpep.tile([48, 512], dt, tag="fle")
                    nc.vector.scalar_tensor_tensor(
                        out=fle[:], in0=fpl[:], scalar=1.0 / LO_SCALE,
                        in1=b2s_sb[:, osl], op0=ALU.mult, op1=ALU.add)
                    fpe = fpep.tile([48, 512], dt, tag="fpe")
                    nc.vector.tensor_add(fpe[:], fle[:], fp[:])
                    nc.sync.dma_start(out=rs_in[g][:], in_=fpe[:])
                    nc.gpsimd.collective_compute(
                        "ReduceScatter",
                        ALU.add,
                        replica_groups=[list(range(NCORES))],
                        ins=[rs_in[g].opt()],
                        outs=[rs_out[g].opt()],
                    )
                    gsl = osl
                    nc.sync.dma_start(out=f_sb[:, gsl], in_=rs_out[g][:])
                    # bf16 hi/lo split of f for mm1, rows [fh,fh,fl,fl]
                    fh = fsp.tile([C, 512], dtb, tag="fh")
                    nc.vector.tensor_copy(fh[:], f_sb[:, gsl])
                    fl32 = fsp.tile([C, 512], dt, tag="fl32")
                    nc.vector.tensor_sub(fl32[:], f_sb[:, gsl], fh[:])
                    fl = fsp.tile([C, 512], dtb, tag="fl")
                    nc.vector.tensor_copy(fl[:], fl32[:])
                    nc.sync.dma_start(out=faug_sb[0:C, gsl], in_=fh[:])
                    nc.sync.dma_start(out=faug_sb[C:2 * C, gsl], in_=fh[:])
                    nc.sync.dma_start(out=faug_sb[2 * C:3 * C, gsl], in_=fl[:])
                    nc.sync.dma_start(out=faug_sb[3 * C:24, gsl], in_=fl[:])
                    # shift[m]: 3-tangent upper bound from u = ||f_m||^2
                    f2 = fsp.tile([C, 512], dtb, tag="f2")
                    nc.vector.tensor_mul(f2[:], f_sb[:, gsl], f_sb[:, gsl])
                    up = mps.tile([P, 4], dt, tag="fp")
                    for j in range(4):
                        nc.tensor.matmul(
                            up[:, j:j + 1],
                            lhsT=f2[:, j * P:(j + 1) * P],
                            rhs=ones6_sb[:],
                            start=True,
                            stop=True,
                        )
                    g4 = slice(g * 4, (g + 1) * 4)
                    t1 = fsp.tile([P, 4], dt, tag="t1")
                    t2 = fsp.tile([P, 4], dt, tag="t2")
                    nc.vector.tensor_mul(t1[:], up[:], shA_sb[:, 0, g4])
                    nc.vector.tensor_add(t1[:], t1[:], shB_sb[:, 0, g4])
                    for i in range(1, NTAN):
                        nc.vector.tensor_mul(t2[:], up[:], shA_sb[:, i, g4])
                        nc.vector.tensor_add(t2[:], t2[:], shB_sb[:, i, g4])
                        nc.vector.tensor_max(t1[:], t1[:], t2[:])
                    nc.vector.tensor_copy(shift_sb[:, g4], t1[:])

                pending = []  # software-pipelined mm2 (quads of m-blocks)

                def emit_mm2_pair(pair):
                    for nt in range(NT):
                        cp = cpsp.tile([C, 512], dt, tag="cp")
                        for i, (mb, et, xp) in enumerate(pair):
                            nc.tensor.matmul(
                                cp[:],
                                lhsT=xp[:].bitcast(dtr),
                                rhs=et[:, nt // 2, nt % 2, :].bitcast(dtr),
                                start=(i == 0),
                                stop=(i == len(pair) - 1),
                            )
                        if first_flush[nt]:
                            nc.vector.tensor_copy(O_nt[nt][:], cp[:])
                            first_flush[nt] = False
                        else:
                            nc.vector.tensor_add(O_nt[nt][:], O_nt[nt][:], cp[:])

                def emit_att_mblock(mb):
                    et = etp.tile([P, 4, 2, 512], dt, tag="et")
                    acc = xpp.tile([P, 4], dt, tag="acc")
                    lhs0 = faug_sb[:, mb * P:(mb + 1) * P]
                    for r in range(4):  # 2 n-chunks per psum tile
                        tps = tpp.tile([P, 2, 512], dt, tag="tps")
                        for i in range(2):
                            nt = 2 * r + i
                            nc.tensor.matmul(
                                tps[:, i, :],
                                lhsT=lhs0,
                                rhs=xaug_sb[:, nt * 512:(nt + 1) * 512],
                                start=True,
                                stop=True,
                            )
                        nc.scalar.activation(
                            et[:, r, :, :].bitcast(dtr),
                            tps[:],
                            AF.Exp,
                            scale=ls_sb[:, mb:mb + 1],
                            bias=shift_sb[:, mb:mb + 1],
                            accum_out=acc[:, r:r + 1],
                        )
                    # Z from the ACT accumulator outputs
                    zz = xpp.tile([P, 1], dt, tag="zz")
                    nc.vector.tensor_reduce(zz[:], acc[:], AX.X, ALU.add)
                    rc = xpp.tile([P, 1], dt, tag="rc")
                    nc.vector.reciprocal(rc[:], zz[:])
                    xp = xpp.tile([P, C], dt, tag="xp")
                    nc.vector.tensor_scalar_mul(
                        xp[:].bitcast(dtr), xtb_sb[:, mb, :], rc[:])
                    pending.append((mb, et, xp))
                    if len(pending) == 5:
                        emit_mm2_pair(pending[:4])
                        del pending[:4]

                MBG = MBS // NGRP
                for g in range(NGRP):
                    emit_mlp2_group(g)
                    if g > 0:
                        for mb in range((g - 1) * MBG, g * MBG):
                            emit_att_mblock(mb)
                for mb in range((NGRP - 1) * MBG, NGRP * MBG):
                    emit_att_mblock(mb)
                while pending:
                    emit_mm2_pair(pending[:4])
                    del pending[:4]

                # stack the 8 [6,512] chunks into [48,512]
                for nt in range(NT):
                    nc.sync.dma_start(
                        out=O48_sb[C * nt:C * nt + C, :].bitcast(dtr),
                        in_=O_nt[nt][:].bitcast(dtr))

            # ===================== LayerNorm over c + output =====================
            with tc.tile_pool(name="lnps", bufs=2, space="PSUM") as lnps, \
                 tc.tile_pool(name="lnrp", bufs=2, space="PSUM") as lnrp, \
                 tc.tile_pool(name="lnsb", bufs=1) as lnsb:
                eps_sb = lnsb.tile([NT, 1], dt, tag="eps")
                nc.vector.memset(eps_sb[:], EPS)
                O2_sb = lnsb.tile([48, 512], dt, tag="O2")
                nc.vector.tensor_mul(O2_sb[:].bitcast(dtr), O48_sb[:], O48_sb[:])
                s_ps = lnps.tile([NT, 512], dt, tag="sps")
                nc.tensor.matmul(
                    s_ps[:], lhsT=blk_sb[:], rhs=O48_sb[:].bitcast(dtr),
                    start=True, stop=True,
                )
                s2_ps = lnps.tile([NT, 512], dt, tag="sps")
                nc.tensor.matmul(
                    s2_ps[:], lhsT=blk_sb[:], rhs=O2_sb[:].bitcast(dtr),
                    start=True, stop=True,
                )
                mean_sb = lnsb.tile([NT, 512], dt, tag="mean")
                nc.vector.tensor_scalar_mul(
                    mean_sb[:].bitcast(dtr), s_ps[:], 1.0 / C)
                ms_sb = lnsb.tile([NT, 512], dt, tag="ms")
                nc.vector.tensor_mul(ms_sb[:], mean_sb[:], mean_sb[:])
                var_sb = lnsb.tile([NT, 512], dt, tag="var")
                nc.vector.tensor_scalar_mul(var_sb[:], s2_ps[:], 1.0 / C)
                nc.vector.tensor_sub(var_sb[:], var_sb[:], ms_sb[:])
                # rstd = exp(-0.5*ln(var+eps)); Ln+Exp share an ACT table set
                lv_sb = lnsb.tile([NT, 512], dt, tag="lv")
                nc.scalar.activation(lv_sb[:], var_sb[:], AF.Ln, bias=eps_sb[:])
                rstd_sb = lnsb.tile([NT, 512], dt, tag="rstd")
                nc.scalar.activation(rstd_sb[:].bitcast(dtr), lv_sb[:],
                                     AF.Exp, scale=-0.5)
                mrep = lnrp.tile([48, 512], dt, tag="mrep")
                nc.tensor.matmul(
                    mrep[:], lhsT=blkT_sb[:], rhs=mean_sb[:].bitcast(dtr),
                    start=True, stop=True,
                )
                rrep = lnrp.tile([48, 512], dt, tag="mrep")
                nc.tensor.matmul(
                    rrep[:], lhsT=blkT_sb[:], rhs=rstd_sb[:].bitcast(dtr),
                    start=True, stop=True,
                )
                on_sb = lnsb.tile([48, 512], dt, tag="on")
                nc.vector.tensor_sub(on_sb[:], O48_sb[:], mrep[:])
                nc.vector.tensor_mul(on_sb[:], on_sb[:], rrep[:])
                nc.vector.tensor_scalar(
                    out=on_sb[:], in0=on_sb[:],
                    scalar1=wb48_sb[:, 0:1], scalar2=wb48_sb[:, 1:2],
                    op0=ALU.mult, op1=ALU.add,
                )
                for nt in range(NT):
                    nc.sync.dma_start(
                        out=out_d[:, nt * 512:(nt + 1) * 512],
                        in_=on_sb[C * nt:C * nt + C, :],
                    )

    nc.compile()
    return nc


def _host_prep(inputs):
    import ml_dtypes
    bf16 = ml_dtypes.bfloat16
    f8 = ml_dtypes.float8_e4m3

    x = np.asarray(inputs["x"], np.float32)
    feature = np.asarray(inputs["feature"], np.float32)
    fc1_w = np.asarray(inputs["fc1_w"], np.float32)
    fc1_b = np.asarray(inputs["fc1_b"], np.float32)
    fc2_w = np.asarray(inputs["fc2_w"], np.float32)
    fc2_b = np.asarray(inputs["fc2_b"], np.float32)
    logit_scale = np.asarray(inputs["logit_scale"], np.float32)
    norm_w = np.asarray(inputs["norm_w"], np.float32)
    norm_b = np.asarray(inputs["norm_b"], np.float32)

    def split_hl(a):
        hi = a.astype(bf16)
        lo = (a - hi.astype(np.float32)).astype(bf16)
        return hi, lo

    def split_hl8(a):
        hi = a.astype(bf16)
        lo = ((a - hi.astype(np.float32)) * LO_SCALE).astype(f8)
        return hi, lo

    w1T = np.ascontiguousarray(fc1_w.T)                      # [f, h]
    w2T = np.ascontiguousarray(fc2_w.T)                      # [h, o]
    featT = np.ascontiguousarray(feature.reshape(B * C, FF).T)   # [f, bc]
    fth, ftl = split_hl(featT)
    featT_b = np.ascontiguousarray(
        np.stack([fth, ftl], axis=1).reshape(KT1, P, 2, B * C)
        .transpose(1, 0, 2, 3))
    featT_8 = np.ascontiguousarray(
        fth.astype(f8).reshape(KT1, P, B * C).transpose(1, 0, 2))
    ls = np.exp(np.minimum(logit_scale.reshape(HW), np.log(np.float32(100.0))))
    ls_b = np.ascontiguousarray(ls.reshape(MBS, P).T).astype(np.float32)
    ones1 = np.ones((1, B * C), np.float32)
    ones6 = np.ones((C, 1), bf16)
    id48 = np.eye(48, dtype=np.float32)
    blk = np.zeros((48, NT), np.float32)
    blk[np.arange(48), np.arange(48) // C] = 1.0
    blkT = np.ascontiguousarray(blk.T)
    wb48 = np.ascontiguousarray(
        np.stack([np.tile(norm_w, NT), np.tile(norm_b, NT)], axis=1))
    b2 = np.tile((fc2_b / NCORES).reshape(1, HW), (48, 1)).astype(np.float32)

    in_maps = []
    for k in range(NCORES):
        w1k = np.ascontiguousarray(w1T[:, k * HS:(k + 1) * HS])   # [9216, 1152]
        w1h, w1l = split_hl8(w1k)
        w1hs = np.ascontiguousarray(
            w1h.reshape(KT1 // 4, 4, P, HS).transpose(0, 2, 1, 3))
        w1ls = np.ascontiguousarray(
            w1l.reshape(KT1 // 8, 8, P, HS).transpose(0, 2, 1, 3))
        b1k = np.ascontiguousarray(fc1_b[k * HS:(k + 1) * HS]).reshape(1, HS)
        w2k = np.ascontiguousarray(w2T[k * HS:(k + 1) * HS, :])   # [1152, 4096]
        w2h, w2l = split_hl8(w2k)
        w2hs = np.ascontiguousarray(w2h.reshape(JT, P, HW).transpose(1, 0, 2))
        w2ls = np.ascontiguousarray(w2l.reshape(JT, P, HW).transpose(1, 0, 2))
        xh, xl = split_hl(x[k])                                   # [6, 4096]
        xaug = np.concatenate([xh, xl, xh, xl], axis=0)           # [24, 4096]

        xtbk = np.ascontiguousarray(x[k].T.reshape(MBS, P, C).transpose(1, 0, 2))
        xmax_k = float(np.linalg.norm(x[k], axis=0).max())
        # tangent-line bounds on ls*xmax*sqrt(u) at u = U_TANGENTS[i]:
        #   bound_i(u) = ls*xmax*(u/sqrt(u_i) + sqrt(u_i))/2
        # shift = max_i(-bound_i(u)) + HEADROOM = max_i(A_i*u + B_i)
        shA = np.zeros((P, NTAN, MBS), np.float32)
        shB = np.zeros((P, NTAN, MBS), np.float32)
        for i, u0 in enumerate(U_TANGENTS):
            r = np.sqrt(np.float32(u0))
            shA[:, i, :] = -ls_b * xmax_k / (2 * r)
            shB[:, i, :] = -ls_b * xmax_k * r / 2 + HEADROOM
        in_maps.append({
            "featT": featT_b, "featT8": featT_8,
            "w1h": w1hs, "w1l": w1ls, "b1": b1k,
            "w2h": w2hs, "w2l": w2ls, "b2": b2,
            "xaug": np.ascontiguousarray(xaug), "xtb": xtbk, "lsb": ls_b, "shA": shA, "shB": shB,
            "ones1": ones1, "ones6": ones6, "id48": id48,
            "blk": blk, "blkT": blkT, "wb48": wb48,
        })
    return in_maps


def _install_ntff_shim():
    # The agent image's `antenv` lacks `axon_hooks`, which bass_utils needs
    # for trace=True under axon. Fabricate the registry module and install
    # the ctypes-based NTFF hook against libaxon_pjrt.so.
    import sys
    import types
    import ctypes
    import contextlib

    try:
        import antenv.axon_hooks  # noqa: F401
        return
    except ImportError:
        pass
    if "antenv.axon_hooks" in sys.modules:
        return
    mod = types.ModuleType("antenv.axon_hooks")
    _h = [None]
    mod.set_axon_ntff_profile_hook = lambda h: _h.__setitem__(0, h)
    mod.get_axon_ntff_profile_hook = lambda: _h[0]
    sys.modules["antenv.axon_hooks"] = mod

    so_path = "/opt/axon/libaxon_pjrt.so"
    if not os.path.exists(so_path):
        return
    lib = ctypes.CDLL(so_path)
    if not hasattr(lib, "axon_start_nrt_profile"):
        return
    lib.axon_start_nrt_profile.argtypes = [
        ctypes.POINTER(ctypes.c_int64), ctypes.c_size_t]
    lib.axon_start_nrt_profile.restype = ctypes.c_int64
    lib.axon_stop_nrt_profile.argtypes = [ctypes.c_char_p]
    lib.axon_stop_nrt_profile.restype = ctypes.c_int64

    @contextlib.contextmanager
    def _hook(output_dir, device_ids):
        import jax
        jax.devices()
        if device_ids:
            ids = (ctypes.c_int64 * len(device_ids))(*device_ids)
            rc = lib.axon_start_nrt_profile(ids, len(device_ids))
        else:
            rc = lib.axon_start_nrt_profile(None, 0)
        if rc != 0:
            raise RuntimeError(f"axon_start_nrt_profile rc={rc}")
        try:
            yield
        finally:
            n = lib.axon_stop_nrt_profile(str(output_dir).encode())
            print(f"ntff profile: {n} file(s) written to {output_dir}")

    mod.set_axon_ntff_profile_hook(_hook)


def kernel(**inputs):
    from concourse.bass_utils import run_bass_kernel_spmd

    if bool(int(os.environ.get("BASS_KT_TRACE", "0"))):
        _install_ntff_shim()

    if "nc" not in _cache:
        _cache["nc"] = _build_program()
    nc = _cache["nc"]

    in_maps = _host_prep(inputs)
    trace = bool(int(os.environ.get("BASS_KT_TRACE", "0")))
    res = run_bass_kernel_spmd(nc, in_maps, list(range(NCORES)), trace=trace)
    kernel.last_results = res
    out = np.stack([np.asarray(res.results[k]["out"]) for k in range(NCORES)])
    return out.astype(np.float32)
